# revision 15
# baseline (speedup 1.0000x reference)
"""Trainium2 Bass kernel for GroupNorm + 4-head self-attention + proj + residual.

Full inputs: x (8, 256, 32, 32), gn_gamma/beta (256,), qkv_w (768, 256),
qkv_b (768,), proj_w (256, 256), proj_b (256,). Output (8, 256, 32, 32).

Sharding: pure data-parallel - one batch element per NeuronCore (8 cores),
weights replicated (host-packed), no collectives.

Per-core dataflow (x as [c=256, hw=1024] stored [128, t=2, 1024]):
  GroupNorm: free-dim reductions on DVE/ACT plus tiny mask matmuls for the
    cross-partition group sums; rsqrt as exp(-0.5*ln(var+eps)) on ACT.
  fp8 everywhere the PE cost matters: qkv/vT/proj matmuls run fp8e4m3
    DoubleRow (2 k-tiles per instruction at 0.5 cyc/col - a 4x cycle
    reduction vs bf16); weights host-scaled x16 into fp8 normal range with
    the descale folded into the PSUM evacuations. S stays bf16 (K=64 per
    head cannot DoubleRow).
  attention: S^T = k^T q per head (softmax axis on PSUM partitions), two
    heads packed per PSUM tile [h0|h1] so one exp covers both; P =
    exp(S - 3) written by ACT directly as fp8 (shift cancels exactly in
    the softmax ratio; keeps exp <= 240 = fp8e4 max); PV accumulates
    U = [vT|1].T @ P via DoubleRow with the ones column giving the
    denominator for free.
  normalization: 1/sumexp via DVE reciprocal (not ACT ln/exp - ACT is the
    bottleneck engine), broadcast with a K=2 f32r head-selector matmul
    (selector = 16.0 to pre-scale attn into fp8 range for proj),
    normalize-on-evacuation straight from U's PSUM into fp8.
  proj: 2 DoubleRow matmuls over both pair halves; residual + 1/256
    descale fused in one scalar_tensor_tensor per output tile.
  DMA: 4 batched input DMAs + 2 output DMAs (issue cost ~700ns each on the
    sync queue dominated the old 20-DMA head).

Known environment quirks handled in _install_patches: this walrus build
allows one sync-wait per instruction (waits are split onto NoOps at the BIR
level) and the Tile exit drain is patched the same way.
"""

import sys

if "/opt/trn_rl_repo" not in sys.path:
    sys.path.insert(0, "/opt/trn_rl_repo")

import ml_dtypes
import numpy as np

import concourse.bass as bass
import concourse.mybir as mybir
from concourse.tile import TileContext
from concourse.bass_utils import run_bass_kernel_spmd

F32 = mybir.dt.float32
F32R = mybir.dt.float32r
BF16 = mybir.dt.bfloat16
F8 = mybir.dt.float8e4
AF = mybir.ActivationFunctionType
OP = mybir.AluOpType
DR = mybir.MatmulPerfMode.DoubleRow

C = 256
HW = 1024
NH = 4
DH = 64
GROUPS = 8
CPG = C // GROUPS
EPS = 1e-5
N_CORES = 8
NT = C // 128

SHIFT = 3.0    # exp(S - SHIFT): keeps exp <= 240 (fp8e4 max); cancels in softmax
WS = 16.0      # weight scale into fp8 normal range
ASCALE = DH ** -0.5

# cst columns
A_QKB = 0      # 0-3: qkv bias per m-tile (q01, k01, q23, k23); q cols pre-scaled
A_PB = 4       # 4-5: proj bias (+ proj_w @ v_bias) per out tile
A_GAM = 6      # 6-7: gn gamma per tile (unused on-chip; folded into gscat)
A_BET = 8      # 8-9: gn beta per tile
A_GSEL = 10    # 10-25: gsel per tile [128, 8] each
A_EXPB = 26    # -SHIFT
A_HSEL = 28    # 28-155: head-selector rows 0/32 (value 16.0)
A_GSC = 156    # 156-283: gscat tile0 at rows 0-7, tile1 at rows 32-39
CSTW = 284


# ---------------------------------------------------------------------------
# Environment patches (walrus in this image allows 1 sync-wait per
# instruction; Tile emits more). Inline so kernel.py is self-contained.
# ---------------------------------------------------------------------------

def _install_patches():
    import orjson
    import concourse.tile as tile_mod
    import concourse.bass2jax as b2j
    import concourse.bass_utils as bu
    from concourse.vector_clock import ScopedClock

    if getattr(tile_mod, "_attn_kernel_patched", False):
        return

    def _drain_and_barrier(self, tick_clock, wait_clock):
        nc = self.nc
        drain_inst = nc.sync.drain()
        wait_clock.add_sem_waits(
            drain_inst.ins, ScopedClock({None: tick_clock.global_clock})
        )
        si = drain_inst.ins.sync_info
        waits = list(si.on_wait or [])
        if len(waits) > 1:
            si.on_wait = waits[:1]
            for j, w in enumerate(waits[1:]):
                nop_inst = nc.sync.nop(nofuse=True)
                nop_inst.ins.sync_info = mybir.SyncInfo(on_wait=[w], on_update=[])
        nc.all_engine_barrier()
        assert self.sems is not None
        popped = nc._tile_sem_poison_stack.pop()
        assert popped is self._sem_poison
        nc.clear_and_free_semaphores(list(self.sems.allocated().values()))
        nc.all_engine_barrier()

    tile_mod.TileContext._drain_and_barrier = _drain_and_barrier

    def _legalize_bir_waits(bir_bytes):
        d = orjson.loads(bir_bytes)
        changed = False
        for fn in d.get("functions", []):
            for bb in fn.get("blocks", []):
                out = []
                for inst in bb.get("instructions", []):
                    si = inst.get("sync_info")
                    waits = (si or {}).get("on_wait") or []
                    if len(waits) > 1:
                        changed = True
                        for j, w in enumerate(waits[:-1]):
                            out.append(
                                {
                                    "debug": inst.get("debug", 0),
                                    "engine": inst["engine"],
                                    "ins": [],
                                    "name": f"{inst['name']}-ws{j}",
                                    "opcode": "NoOp",
                                    "outs": [],
                                    "sync_info": {"on_update": [], "on_wait": [w]},
                                }
                            )
                        si["on_wait"] = [waits[-1]]
                    out.append(inst)
                bb["instructions"] = out
        return orjson.dumps(d) if changed else bir_bytes

    orig_compile = b2j.compile_bir_kernel

    def _compile_wrapper(ant_bir_str, *args, **kwargs):
        return orig_compile(_legalize_bir_waits(ant_bir_str), *args, **kwargs)

    b2j.compile_bir_kernel = _compile_wrapper
    bu.upload_artifacts = lambda tmpdir: "local://" + tmpdir
    tile_mod._attn_kernel_patched = True


# ---------------------------------------------------------------------------
# Kernel graph (SPMD, per core)
# ---------------------------------------------------------------------------

def build_nc():
    nc = bass.Bass()
    x_ext = nc.declare_dram_parameter("x", [128, 2 * HW], F32, isOutput=False)
    w8_ext = nc.declare_dram_parameter("w8", [128, 2048], F8, isOutput=False)
    cst_ext = nc.declare_dram_parameter("cst", [128, CSTW], F32, isOutput=False)
    out_ext = nc.declare_dram_parameter("out", [128, 2 * HW], F32, isOutput=True)

    with TileContext(nc) as tc:
        with (
            tc.tile_pool(name="const", bufs=1) as cpool,
            tc.tile_pool(name="xp", bufs=1) as xpool,
            tc.tile_pool(name="work", bufs=1) as wpool,
            tc.tile_pool(name="pexp", bufs=3) as ppool,
            tc.tile_pool(name="small", bufs=1) as spool,
            tc.tile_pool(name="psM", bufs=2, space="PSUM") as psM,
            tc.tile_pool(name="psU", bufs=2, space="PSUM") as psU,
        ):
            # ---------------- input DMAs (batched) ----------------
            x2 = xpool.tile([128, 2 * HW], F32, name="x2", tag="x2")
            for c4 in range(4):
                nc.sync.dma_start(
                    out=x2[:, c4 * 512:(c4 + 1) * 512],
                    in_=x_ext[:, c4 * 512:(c4 + 1) * 512],
                )
            cst = cpool.tile([128, CSTW], F32, name="cst", tag="cst")
            nc.sync.dma_start(out=cst[:], in_=cst_ext[:])
            w8 = cpool.tile([128, 2048], F8, name="w8", tag="w8")
            nc.sync.dma_start(out=w8[:], in_=w8_ext[:])

            # ---------------- GN moments ----------------
            stats_f = spool.tile([128, 8], F32R, name="stats", tag="stats")
            for c4 in range(4):
                with nc.allow_low_precision(reason="fp32r moment accum"):
                    nc.vector.tensor_reduce(
                        out=stats_f[:, 2 * c4:2 * c4 + 1],
                        in_=x2[:, c4 * 512:(c4 + 1) * 512],
                        op=OP.add, axis=mybir.AxisListType.X,
                    )
                sq_scr = wpool.tile([128, 512], F32, name=f"sq{c4}", tag=f"sq{c4}")
                with nc.allow_low_precision(reason="fp32r moment accum"):
                    nc.scalar.activation(
                        sq_scr[:], x2[:, c4 * 512:(c4 + 1) * 512], AF.Square,
                        accum_out=stats_f[:, 2 * c4 + 1:2 * c4 + 2],
                    )

            # ---------------- f32r const copies ----------------
            gsel_r = [cpool.tile([128, GROUPS], F32R, name=f"gselr{t}", tag=f"gselr{t}")
                      for t in range(NT)]
            gscat_r = [cpool.tile([GROUPS, 128], F32R, name=f"gscatr{t}", tag=f"gscatr{t}")
                       for t in range(NT)]
            hsel_r = cpool.tile([33, 128], F32R, name="hselr", tag="hselr")
            for t in range(NT):
                nc.vector.tensor_copy(
                    gsel_r[t][:], cst[:, A_GSEL + 8 * t:A_GSEL + 8 * (t + 1)]
                )
                # tile1 parked at rows 32-39 (engine partition offsets must be
                # multiples of 32)
                nc.vector.tensor_copy(
                    gscat_r[t][:], cst[32 * t:32 * t + 8, A_GSC:A_GSC + 128]
                )
            nc.vector.tensor_copy(hsel_r[:], cst[0:33, A_HSEL:A_HSEL + 128])

            # ---------------- GN stats -> per-channel affine ----------------
            gstat_ps = psM.tile([GROUPS, 2], F32, name="gstat", tag="ps")
            for c4 in range(4):
                nc.tensor.matmul(
                    gstat_ps[:], gsel_r[c4 // 2][:], stats_f[:, 2 * c4:2 * c4 + 2],
                    start=(c4 == 0), stop=(c4 == 3),
                )
            gstat_sb = spool.tile([GROUPS, 2], F32, name="gstat_sb", tag="gstat_sb")
            nc.vector.tensor_copy(gstat_sb[:], gstat_ps[:])
            eps_ap = spool.tile([GROUPS, 1], F32, name="epsap", tag="epsap")
            nc.gpsimd.memset(eps_ap[:], EPS)
            m2 = spool.tile([GROUPS, 1], F32, name="m2", tag="m2")
            nc.vector.tensor_tensor(m2[:], gstat_sb[:, 0:1], gstat_sb[:, 0:1], OP.mult)
            var = spool.tile([GROUPS, 1], F32, name="var", tag="var")
            nc.vector.tensor_tensor(var[:], gstat_sb[:, 1:2], m2[:], OP.subtract)
            lnv = spool.tile([GROUPS, 1], F32, name="lnv", tag="lnv")
            nc.scalar.activation(lnv[:], var[:], AF.Ln, bias=eps_ap[:, 0:1])
            rs2 = spool.tile([GROUPS, 2], F32R, name="rs2", tag="rs2")
            nc.scalar.activation(rs2[:, 0:1], lnv[:], AF.Exp, scale=-0.5)
            nc.vector.tensor_tensor(rs2[:, 1:2], gstat_sb[:, 0:1], rs2[:, 0:1], OP.mult)

            xn2 = wpool.tile([128, 2 * HW], F8, name="xn2", tag="xn2")
            for t in range(NT):
                chan_ps = psM.tile([128, 2], F32, name="chan", tag="ps")
                nc.tensor.matmul(chan_ps[:], gscat_r[t][:], rs2[:], start=True, stop=True)
                # gscat rows pre-scaled by gamma on host: chan_ps already
                # holds [gamma*rsqrt, gamma*mean*rsqrt]
                nB_sb = spool.tile([128, 1], F32, name=f"nB{t}", tag=f"nB{t}")
                nc.vector.tensor_scalar(
                    nB_sb[:], chan_ps[:, 1:2],
                    cst[:, A_BET + t:A_BET + t + 1], None, OP.subtract,
                )
                nc.vector.tensor_scalar(
                    xn2[:, t * HW:(t + 1) * HW], x2[:, t * HW:(t + 1) * HW],
                    chan_ps[:, 0:1], nB_sb[:, 0:1], OP.mult, OP.subtract,
                )

            xn3 = xn2[:].rearrange("p (k c) -> p k c", k=2)

            # ---------------- q, k (fp8 DoubleRow matmul, bf16 out) --------
            # m order: 0=q heads01, 1=k heads01, 2=q heads23, 3=k23.
            # S itself stays bf16: this device's power governor slows fp8
            # dual-pump matmuls to ~1.23ns/col vs bf16's ~0.89, so fp8 S
            # (which cannot halve the instruction count at K=64) loses.
            # q/k carry no 1/8 attention scale - that folds into exp's scale.
            qkw = w8[:, 0:1024].rearrange("p (k j) -> p k j", k=2)
            qk_sb = [wpool.tile([128, HW], BF16, name=f"qk{m}", tag=f"qk{m}")
                     for m in range(4)]
            for m in range(4):
                pool, tag = (psM, "ps") if m % 2 == 0 else (psU, "u")
                mm_ps = pool.tile([128, HW], F32, name="qkvp", tag=tag)
                for ch in range(2):
                    nc.tensor.matmul(
                        mm_ps[:, ch * 512:(ch + 1) * 512],
                        qkw[:, :, m * 128:(m + 1) * 128],
                        xn3[:, :, ch * 512:(ch + 1) * 512],
                        start=True, stop=True, perf_mode=DR,
                    )
                if m == 0:
                    # m0 on ACT, m1 on DVE: the first S block needs both, so
                    # they evacuate in parallel
                    nc.scalar.activation(
                        qk_sb[m][:], mm_ps[:], AF.Identity,
                        bias=cst[:, A_QKB + m:A_QKB + m + 1], scale=1.0 / WS,
                    )
                else:
                    nc.vector.tensor_scalar(
                        qk_sb[m][:], mm_ps[:], 1.0 / WS,
                        cst[:, A_QKB + m:A_QKB + m + 1], OP.mult, OP.add,
                    )

            # ---------------- vT (fp8 DoubleRow, ones column per head) -----
            # 272 = 4 heads x 68: the DoubleRow ldweights k-tile stride must be
            # a multiple of 16 elements (s3_lw_dual_fp8_restrictions)
            vt_sb = wpool.tile([128, 8 * 272], F8, name="vt", tag="vt")
            vt4 = vt_sb[:].rearrange("p (e h c) -> p e h c", e=8, h=4, c=68)
            nc.vector.tensor_scalar(
                vt4[:, :, :, 64:65],
                x2[:, 0:32].rearrange("p (a b c) -> p a b c", a=8, b=4),
                0.0, 1.0, OP.mult, OP.add,
            )
            vw = w8[:, 1024:1536].rearrange("p (k c) -> p k c", k=2)
            for et in range(8):
                vt_ps = psU.tile([128, C], F32, name="vtp", tag="u")
                nc.tensor.matmul(
                    vt_ps[:], xn3[:, :, et * 128:(et + 1) * 128], vw[:],
                    start=True, stop=True, perf_mode=DR,
                )
                nc.vector.tensor_scalar(
                    vt4[:, et, :, 0:64],
                    vt_ps[:].rearrange("p (h c) -> p h c", h=4),
                    1.0 / WS, None, OP.mult,
                )

            # residual prep (x + proj_bias), off the critical path
            resid2 = wpool.tile([128, 2 * HW], F32, name="resid", tag="resid")
            for t in range(NT):
                nc.vector.tensor_scalar(
                    resid2[:, t * HW:(t + 1) * HW], x2[:, t * HW:(t + 1) * HW],
                    cst[:, A_PB + t:A_PB + t + 1], None, OP.add,
                )

            # ---------------- attention ----------------
            attn2 = wpool.tile([128, 2 * HW], F8, name="attn2", tag="attn2")
            srows = [spool.tile([33, HW], F32, name=f"srows{p}", tag=f"srows{p}")
                     for p in range(2)]
            lnp = [spool.tile([33, HW], F32, name=f"lnp{p}", tag=f"lnp{p}")
                   for p in range(2)]
            srecr = [spool.tile([33, HW], F32R, name=f"srecr{p}", tag=f"srecr{p}")
                     for p in range(2)]
            rb_sb = [wpool.tile([128, HW], BF16, name=f"rb{p}", tag=f"rb{p}")
                     for p in range(2)]
            u_ps_pair = [{}, {}]

            # rows 1-31 preset to 1.0: the K=33 rb matmul contracts them
            # against zero selector rows, so they must not be inf/nan
            for p in range(2):
                nc.gpsimd.memset(srows[p][:], 1.0)

            def emit_denom_dve(pair, hh=None):
                # denominator -> 1/s = exp(-ln(s)): DVE reciprocal costs
                # 6.5us/call (iterative ucode) and the custom-DVE approx ops
                # don't codegen on this walrus build, so ACT ln/exp it is -
                # it sits at pair boundaries where the exp stream has slack.
                # hh=0/1 processes one head's row (tail chain: h0's starts
                # while h1's last PV runs); hh=None does both in one pass.
                u_ps = u_ps_pair[pair]
                rows = (0, 1) if hh is None else (hh,)
                for r in rows:
                    nc.vector.tensor_copy(
                        srows[pair][32 * r:32 * r + 1, :], u_ps[r][64:65, :]
                    )
                if hh is None:
                    sl = slice(0, 33)
                else:
                    sl = slice(32 * hh, 32 * hh + 1)
                nc.scalar.activation(lnp[pair][sl, :], srows[pair][sl, :], AF.Ln)
                nc.scalar.activation(
                    srecr[pair][sl, :], lnp[pair][sl, :], AF.Exp, scale=-1.0
                )

            def emit_rb_chain(pair):
                # broadcast 16/s across partitions, then normalize straight
                # out of U's PSUM into fp8 (x16 for proj's fp8 range)
                rb_ps = psM.tile([128, HW], F32, name="rb", tag="ps")
                u_ps = u_ps_pair[pair]
                # per-ch pipeline: ch0's cast+normalize runs while ch1's rb
                # matmul is still on the PE
                for ch in range(2):
                    nc.tensor.matmul(
                        rb_ps[:, ch * 512:(ch + 1) * 512],
                        hsel_r[:],
                        srecr[pair][:, ch * 512:(ch + 1) * 512],
                        start=True, stop=True,
                    )
                    nc.vector.tensor_copy(
                        rb_sb[pair][:, ch * 512:(ch + 1) * 512],
                        rb_ps[:, ch * 512:(ch + 1) * 512],
                    )
                    for hh in range(2):
                        nc.vector.tensor_tensor(
                            attn2[hh * 64:(hh + 1) * 64,
                                  pair * HW + ch * 512:pair * HW + (ch + 1) * 512],
                            u_ps[hh][0:64, ch * 512:(ch + 1) * 512],
                            rb_sb[pair][hh * 64:(hh + 1) * 64, ch * 512:(ch + 1) * 512],
                            OP.mult,
                        )

            for pair in range(2):
                qt = qk_sb[2 * pair]
                kt_t = qk_sb[2 * pair + 1]
                u_ps = u_ps_pair[pair]

                def emit_pv(j, ptv, only_hh=None):
                    for hh in range(2) if only_hh is None else (only_hh,):
                        if hh not in u_ps:
                            u_ps[hh] = psU.tile([65, HW], F32, name=f"u{hh}", tag="u")
                        for ch in range(2):
                            nc.tensor.matmul(
                                u_ps[hh][:, ch * 512:(ch + 1) * 512],
                                vt4[:, 2 * j:2 * j + 2, 2 * pair + hh, 0:65],
                                ptv[:, :, hh, ch, :],
                                start=(j == 0), stop=(j == 3),
                                perf_mode=DR, skip_group_check=True,
                            )

                pending = []
                for j in range(4):
                    pt = ppool.tile([128, 4096], F8, name="p", tag="p")
                    ptv = pt[:].rearrange("p (i h c q) -> p i h c q", i=2, h=2, c=2)
                    for i in range(2):
                        et = 2 * j + i
                        for ch in range(2):
                            s_ps = psM.tile([128, HW], F32, name="s", tag="ps")
                            for hh in range(2):
                                prow = slice(hh * 64, hh * 64 + 64)
                                nc.tensor.matmul(
                                    s_ps[:, hh * 512:(hh + 1) * 512],
                                    kt_t[prow, et * 128:(et + 1) * 128],
                                    qt[prow, ch * 512:(ch + 1) * 512],
                                    start=True, stop=True,
                                )
                            # scale = the 1/8 attention scale left out of q/k
                            nc.scalar.activation(
                                ptv[:, i, 0:2, ch, :], s_ps[:], AF.Exp,
                                bias=cst[:, A_EXPB:A_EXPB + 1], scale=ASCALE,
                            )
                    if pair == 1 and j == 1:
                        # pair0's normalize lands here: after two of pair1's S
                        # blocks (PE is in-order; earlier would stall it on
                        # the DVE reciprocal chain) yet before the first PV
                        # needs pair0's U banks
                        emit_rb_chain(0)
                    if pending:
                        emit_pv(*pending.pop(0))
                    pending.append((j, ptv))
                while pending:
                    j, ptv = pending.pop(0)
                    if pending:
                        emit_pv(j, ptv)
                    elif pair == 1:
                        # final j of the last pair: h0's PVs, then its
                        # denominator chain while h1's PVs still run on the
                        # PE (shortens the serial tail)
                        emit_pv(j, ptv, only_hh=0)
                        emit_denom_dve(pair, hh=0)
                        emit_pv(j, ptv, only_hh=1)
                        emit_denom_dve(pair, hh=1)
                    else:
                        # pair0's rb chain waits for pair1's S blocks anyway;
                        # one combined [33, x] ln/exp costs the ACT stream
                        # less than two per-head passes
                        emit_pv(j, ptv)
                        emit_denom_dve(pair)
            emit_rb_chain(1)

            # ---------------- proj (fp8 DoubleRow) + residual ----------------
            pw = w8[:, 1536:2048].rearrange("p (k c) -> p k c", k=2)
            at3 = attn2[:].rearrange("p (i q) -> p i q", i=2)
            out_sb = wpool.tile([128, 2 * HW], F32, name="out", tag="out")
            for t in range(NT):
                pp = psM.tile([128, HW], F32, name=f"pp{t}", tag="ps")
                for ch in range(2):
                    nc.tensor.matmul(
                        pp[:, ch * 512:(ch + 1) * 512],
                        pw[:, :, t * 128:(t + 1) * 128],
                        at3[:, :, ch * 512:(ch + 1) * 512],
                        start=True, stop=True, perf_mode=DR,
                    )
                nc.vector.scalar_tensor_tensor(
                    out_sb[:, t * HW:(t + 1) * HW], pp[:], 1.0 / 256.0,
                    resid2[:, t * HW:(t + 1) * HW], OP.mult, OP.add,
                )
                nc.sync.dma_start(
                    out=out_ext[:, t * HW:(t + 1) * HW],
                    in_=out_sb[:, t * HW:(t + 1) * HW],
                )
    return nc


# ---------------------------------------------------------------------------
# Host entry point
# ---------------------------------------------------------------------------

_CACHED_NC = None


def _host_inputs(x, gn_gamma, gn_beta, qkv_w, qkv_b, proj_w, proj_b):
    f32 = np.float32
    qw = np.asarray(qkv_w[0:C], f32)
    kw = np.asarray(qkv_w[C:2 * C], f32)
    vw = np.asarray(qkv_w[2 * C:3 * C], f32)
    qb = np.asarray(qkv_b[0:C], f32)
    kb = np.asarray(qkv_b[C:2 * C], f32)
    vb = np.asarray(qkv_b[2 * C:3 * C], f32)
    pw = np.asarray(proj_w, f32)
    pb = np.asarray(proj_b, f32)

    def pack_dr(w):  # [rows, C] -> [128, 2*rows] with [p, kt, row] layout
        r = w.shape[0]
        return (w.T * WS).reshape(2, 128, r).transpose(1, 0, 2).reshape(128, 2 * r)

    w_all = np.concatenate([qw[0:128], kw[0:128], qw[128:256], kw[128:256]], axis=0)
    w8 = np.concatenate(
        [pack_dr(w_all), pack_dr(vw), pack_dr(pw)], axis=1
    ).astype(ml_dtypes.float8_e4m3)  # [128, 2048]

    qkb = np.stack([qb[0:128], kb[0:128], qb[128:256], kb[128:256]])
    pb_eff = (pb + pw @ vb).reshape(2, 128)
    gam = np.asarray(gn_gamma, f32).reshape(2, 128)
    bet = np.asarray(gn_beta, f32).reshape(2, 128)

    cst = np.zeros((128, CSTW), f32)
    cst[:, A_QKB:A_QKB + 4] = qkb.T
    cst[:, A_PB:A_PB + 2] = pb_eff.T
    cst[:, A_GAM:A_GAM + 2] = gam.T
    cst[:, A_BET:A_BET + 2] = bet.T
    cidx = np.arange(C)
    gsel = (cidx[:, None] // CPG == np.arange(GROUPS)[None, :]).astype(f32) / (CPG * HW)
    cst[:, A_GSEL:A_GSEL + 8] = gsel[0:128]
    cst[:, A_GSEL + 8:A_GSEL + 16] = gsel[128:256]
    cst[:, A_EXPB] = -SHIFT
    cst[0, A_HSEL:A_HSEL + 64] = WS
    cst[32, A_HSEL + 64:A_HSEL + 128] = WS
    mask = (cidx[:, None] // CPG == np.arange(GROUPS)[None, :]).astype(f32)
    gamv = np.asarray(gn_gamma, f32)
    cst[0:8, A_GSC:A_GSC + 128] = (mask[0:128] * gamv[0:128, None]).T
    cst[32:40, A_GSC:A_GSC + 128] = (mask[128:256] * gamv[128:256, None]).T

    shared = {"w8": w8, "cst": np.ascontiguousarray(cst)}
    x = np.asarray(x, f32)
    in_maps = []
    for i in range(N_CORES):
        m = dict(shared)
        m["x"] = np.ascontiguousarray(
            x[i].reshape(2, 128, HW).transpose(1, 0, 2).reshape(128, 2 * HW)
        )
        in_maps.append(m)
    return in_maps


def kernel(x, gn_gamma, gn_beta, qkv_w, qkv_b, proj_w, proj_b):
    global _CACHED_NC
    _install_patches()
    in_maps = _host_inputs(x, gn_gamma, gn_beta, qkv_w, qkv_b, proj_w, proj_b)
    if _CACHED_NC is None:
        _CACHED_NC = build_nc()
    res = run_bass_kernel_spmd(_CACHED_NC, in_maps, core_ids=list(range(N_CORES)))
    out = np.stack([
        res.results[i]["out"].reshape(128, 2, HW).transpose(1, 0, 2).reshape(C, 32, 32)
        for i in range(N_CORES)
    ])
    return out.astype(np.float32)


# revision 16
# speedup vs baseline: 1.0139x; 1.0139x over previous
"""Trainium2 Bass kernel for GroupNorm + 4-head self-attention + proj + residual.

Full inputs: x (8, 256, 32, 32), gn_gamma/beta (256,), qkv_w (768, 256),
qkv_b (768,), proj_w (256, 256), proj_b (256,). Output (8, 256, 32, 32).

Sharding: pure data-parallel - one batch element per NeuronCore (8 cores),
weights replicated (host-packed), no collectives.

Per-core dataflow (x as [c=256, hw=1024] stored [128, t=2, 1024]):
  GroupNorm: free-dim reductions on DVE/ACT plus tiny mask matmuls for the
    cross-partition group sums; rsqrt as exp(-0.5*ln(var+eps)) on ACT.
  fp8 everywhere the PE cost matters: qkv/vT/proj matmuls run fp8e4m3
    DoubleRow (2 k-tiles per instruction at 0.5 cyc/col - a 4x cycle
    reduction vs bf16); weights host-scaled x16 into fp8 normal range with
    the descale folded into the PSUM evacuations. S stays bf16 (K=64 per
    head cannot DoubleRow).
  attention: S^T = k^T q per head (softmax axis on PSUM partitions), two
    heads packed per PSUM tile [h0|h1] so one exp covers both; P =
    exp(S - 3) written by ACT directly as fp8 (shift cancels exactly in
    the softmax ratio; keeps exp <= 240 = fp8e4 max); PV accumulates
    U = [vT|1].T @ P via DoubleRow with the ones column giving the
    denominator for free.
  normalization: 1/sumexp via DVE reciprocal (not ACT ln/exp - ACT is the
    bottleneck engine), broadcast with a K=2 f32r head-selector matmul
    (selector = 16.0 to pre-scale attn into fp8 range for proj),
    normalize-on-evacuation straight from U's PSUM into fp8.
  proj: 2 DoubleRow matmuls over both pair halves; residual + 1/256
    descale fused in one scalar_tensor_tensor per output tile.
  DMA: 4 batched input DMAs + 2 output DMAs (issue cost ~700ns each on the
    sync queue dominated the old 20-DMA head).

Known environment quirks handled in _install_patches: this walrus build
allows one sync-wait per instruction (waits are split onto NoOps at the BIR
level) and the Tile exit drain is patched the same way.
"""

import sys

if "/opt/trn_rl_repo" not in sys.path:
    sys.path.insert(0, "/opt/trn_rl_repo")

import ml_dtypes
import numpy as np

import concourse.bass as bass
import concourse.mybir as mybir
from concourse.tile import TileContext
from concourse.bass_utils import run_bass_kernel_spmd

F32 = mybir.dt.float32
F32R = mybir.dt.float32r
BF16 = mybir.dt.bfloat16
F8 = mybir.dt.float8e4
AF = mybir.ActivationFunctionType
OP = mybir.AluOpType
DR = mybir.MatmulPerfMode.DoubleRow

C = 256
HW = 1024
NH = 4
DH = 64
GROUPS = 8
CPG = C // GROUPS
EPS = 1e-5
N_CORES = 8
NT = C // 128

SHIFT = 3.0    # exp(S - SHIFT): keeps exp <= 240 (fp8e4 max); cancels in softmax
WS = 16.0      # weight scale into fp8 normal range
ASCALE = DH ** -0.5

# cst columns
A_QKB = 0      # 0-3: qkv bias per m-tile (q01, k01, q23, k23); q cols pre-scaled
A_PB = 4       # 4-5: proj bias (+ proj_w @ v_bias) per out tile
A_GAM = 6      # 6-7: gn gamma per tile (unused on-chip; folded into gscat)
A_BET = 8      # 8-9: gn beta per tile
A_GSEL = 10    # 10-25: gsel per tile [128, 8] each
A_EXPB = 26    # -SHIFT
A_HSEL = 28    # 28-155: head-selector rows 0/32 (value 16.0)
A_GSC = 156    # 156-283: gscat tile0 at rows 0-7, tile1 at rows 32-39
CSTW = 284


# ---------------------------------------------------------------------------
# Environment patches (walrus in this image allows 1 sync-wait per
# instruction; Tile emits more). Inline so kernel.py is self-contained.
# ---------------------------------------------------------------------------

def _install_patches():
    import orjson
    import concourse.tile as tile_mod
    import concourse.bass2jax as b2j
    import concourse.bass_utils as bu
    from concourse.vector_clock import ScopedClock

    if getattr(tile_mod, "_attn_kernel_patched", False):
        return

    def _drain_and_barrier(self, tick_clock, wait_clock):
        nc = self.nc
        drain_inst = nc.sync.drain()
        wait_clock.add_sem_waits(
            drain_inst.ins, ScopedClock({None: tick_clock.global_clock})
        )
        si = drain_inst.ins.sync_info
        waits = list(si.on_wait or [])
        if len(waits) > 1:
            si.on_wait = waits[:1]
            for j, w in enumerate(waits[1:]):
                nop_inst = nc.sync.nop(nofuse=True)
                nop_inst.ins.sync_info = mybir.SyncInfo(on_wait=[w], on_update=[])
        nc.all_engine_barrier()
        assert self.sems is not None
        popped = nc._tile_sem_poison_stack.pop()
        assert popped is self._sem_poison
        nc.clear_and_free_semaphores(list(self.sems.allocated().values()))
        nc.all_engine_barrier()

    tile_mod.TileContext._drain_and_barrier = _drain_and_barrier

    def _legalize_bir_waits(bir_bytes):
        d = orjson.loads(bir_bytes)
        changed = False
        for fn in d.get("functions", []):
            for bb in fn.get("blocks", []):
                out = []
                for inst in bb.get("instructions", []):
                    si = inst.get("sync_info")
                    waits = (si or {}).get("on_wait") or []
                    if len(waits) > 1:
                        changed = True
                        for j, w in enumerate(waits[:-1]):
                            out.append(
                                {
                                    "debug": inst.get("debug", 0),
                                    "engine": inst["engine"],
                                    "ins": [],
                                    "name": f"{inst['name']}-ws{j}",
                                    "opcode": "NoOp",
                                    "outs": [],
                                    "sync_info": {"on_update": [], "on_wait": [w]},
                                }
                            )
                        si["on_wait"] = [waits[-1]]
                    out.append(inst)
                bb["instructions"] = out
        return orjson.dumps(d) if changed else bir_bytes

    orig_compile = b2j.compile_bir_kernel

    def _compile_wrapper(ant_bir_str, *args, **kwargs):
        return orig_compile(_legalize_bir_waits(ant_bir_str), *args, **kwargs)

    b2j.compile_bir_kernel = _compile_wrapper
    bu.upload_artifacts = lambda tmpdir: "local://" + tmpdir
    tile_mod._attn_kernel_patched = True


# ---------------------------------------------------------------------------
# Kernel graph (SPMD, per core)
# ---------------------------------------------------------------------------

def build_nc():
    nc = bass.Bass()
    x_ext = nc.declare_dram_parameter("x", [128, 2 * HW], F32, isOutput=False)
    w8_ext = nc.declare_dram_parameter("w8", [128, 2048], F8, isOutput=False)
    cst_ext = nc.declare_dram_parameter("cst", [128, CSTW], F32, isOutput=False)
    out_ext = nc.declare_dram_parameter("out", [128, 2 * HW], F32, isOutput=True)

    with TileContext(nc) as tc:
        with (
            tc.tile_pool(name="const", bufs=1) as cpool,
            tc.tile_pool(name="xp", bufs=1) as xpool,
            tc.tile_pool(name="work", bufs=1) as wpool,
            tc.tile_pool(name="pexp", bufs=3) as ppool,
            tc.tile_pool(name="small", bufs=1) as spool,
            tc.tile_pool(name="psM", bufs=2, space="PSUM") as psM,
            tc.tile_pool(name="psU", bufs=2, space="PSUM") as psU,
        ):
            # ---------------- input DMAs (batched) ----------------
            x2 = xpool.tile([128, 2 * HW], F32, name="x2", tag="x2")
            for c4 in range(4):
                nc.sync.dma_start(
                    out=x2[:, c4 * 512:(c4 + 1) * 512],
                    in_=x_ext[:, c4 * 512:(c4 + 1) * 512],
                )
            cst = cpool.tile([128, CSTW], F32, name="cst", tag="cst")
            nc.sync.dma_start(out=cst[:], in_=cst_ext[:])
            w8 = cpool.tile([128, 2048], F8, name="w8", tag="w8")
            nc.sync.dma_start(out=w8[:], in_=w8_ext[:])

            # ---------------- GN moments ----------------
            stats_f = spool.tile([128, 8], F32R, name="stats", tag="stats")
            for c4 in range(4):
                with nc.allow_low_precision(reason="fp32r moment accum"):
                    nc.vector.tensor_reduce(
                        out=stats_f[:, 2 * c4:2 * c4 + 1],
                        in_=x2[:, c4 * 512:(c4 + 1) * 512],
                        op=OP.add, axis=mybir.AxisListType.X,
                    )
                sq_scr = wpool.tile([128, 512], F32, name=f"sq{c4}", tag=f"sq{c4}")
                with nc.allow_low_precision(reason="fp32r moment accum"):
                    nc.scalar.activation(
                        sq_scr[:], x2[:, c4 * 512:(c4 + 1) * 512], AF.Square,
                        accum_out=stats_f[:, 2 * c4 + 1:2 * c4 + 2],
                    )

            # ---------------- f32r const copies ----------------
            gsel_r = [cpool.tile([128, GROUPS], F32R, name=f"gselr{t}", tag=f"gselr{t}")
                      for t in range(NT)]
            gscat_r = [cpool.tile([GROUPS, 128], F32R, name=f"gscatr{t}", tag=f"gscatr{t}")
                       for t in range(NT)]
            hsel_r = cpool.tile([33, 128], F32R, name="hselr", tag="hselr")
            for t in range(NT):
                nc.vector.tensor_copy(
                    gsel_r[t][:], cst[:, A_GSEL + 8 * t:A_GSEL + 8 * (t + 1)]
                )
                # tile1 parked at rows 32-39 (engine partition offsets must be
                # multiples of 32)
                nc.vector.tensor_copy(
                    gscat_r[t][:], cst[32 * t:32 * t + 8, A_GSC:A_GSC + 128]
                )
            nc.vector.tensor_copy(hsel_r[:], cst[0:33, A_HSEL:A_HSEL + 128])

            # ---------------- GN stats -> per-channel affine ----------------
            gstat_ps = psM.tile([GROUPS, 2], F32, name="gstat", tag="ps")
            for c4 in range(4):
                nc.tensor.matmul(
                    gstat_ps[:], gsel_r[c4 // 2][:], stats_f[:, 2 * c4:2 * c4 + 2],
                    start=(c4 == 0), stop=(c4 == 3),
                )
            gstat_sb = spool.tile([GROUPS, 2], F32, name="gstat_sb", tag="gstat_sb")
            nc.vector.tensor_copy(gstat_sb[:], gstat_ps[:])
            eps_ap = spool.tile([GROUPS, 1], F32, name="epsap", tag="epsap")
            nc.gpsimd.memset(eps_ap[:], EPS)
            # negvar = mean^2 - E[x^2] in one op; Ln's scale=-1 flips it back
            nvar = spool.tile([GROUPS, 1], F32, name="nvar", tag="nvar")
            nc.vector.scalar_tensor_tensor(
                nvar[:], gstat_sb[:, 0:1], gstat_sb[:, 0:1], gstat_sb[:, 1:2],
                OP.mult, OP.subtract,
            )
            lnv = spool.tile([GROUPS, 1], F32, name="lnv", tag="lnv")
            nc.scalar.activation(lnv[:], nvar[:], AF.Ln, bias=eps_ap[:, 0:1], scale=-1.0)
            rs2 = spool.tile([GROUPS, 2], F32R, name="rs2", tag="rs2")
            nc.scalar.activation(rs2[:, 0:1], lnv[:], AF.Exp, scale=-0.5)
            nc.vector.tensor_tensor(rs2[:, 1:2], gstat_sb[:, 0:1], rs2[:, 0:1], OP.mult)

            xn2 = wpool.tile([128, 2 * HW], F8, name="xn2", tag="xn2")
            for t in range(NT):
                chan_ps = psM.tile([128, 2], F32, name="chan", tag="ps")
                nc.tensor.matmul(chan_ps[:], gscat_r[t][:], rs2[:], start=True, stop=True)
                # gscat rows pre-scaled by gamma on host: chan_ps already
                # holds [gamma*rsqrt, gamma*mean*rsqrt]
                nB_sb = spool.tile([128, 1], F32, name=f"nB{t}", tag=f"nB{t}")
                nc.vector.tensor_scalar(
                    nB_sb[:], chan_ps[:, 1:2],
                    cst[:, A_BET + t:A_BET + t + 1], None, OP.subtract,
                )
                nc.vector.tensor_scalar(
                    xn2[:, t * HW:(t + 1) * HW], x2[:, t * HW:(t + 1) * HW],
                    chan_ps[:, 0:1], nB_sb[:, 0:1], OP.mult, OP.subtract,
                )

            xn3 = xn2[:].rearrange("p (k c) -> p k c", k=2)

            # ---------------- q, k (fp8 DoubleRow matmul, bf16 out) --------
            # m order: 0=q heads01, 1=k heads01, 2=q heads23, 3=k23.
            # S itself stays bf16: this device's power governor slows fp8
            # dual-pump matmuls to ~1.23ns/col vs bf16's ~0.89, so fp8 S
            # (which cannot halve the instruction count at K=64) loses.
            # q/k carry no 1/8 attention scale - that folds into exp's scale.
            qkw = w8[:, 0:1024].rearrange("p (k j) -> p k j", k=2)
            qk_sb = [wpool.tile([128, HW], BF16, name=f"qk{m}", tag=f"qk{m}")
                     for m in range(4)]
            for m in range(4):
                pool, tag = (psM, "ps") if m % 2 == 0 else (psU, "u")
                mm_ps = pool.tile([128, HW], F32, name="qkvp", tag=tag)
                for ch in range(2):
                    nc.tensor.matmul(
                        mm_ps[:, ch * 512:(ch + 1) * 512],
                        qkw[:, :, m * 128:(m + 1) * 128],
                        xn3[:, :, ch * 512:(ch + 1) * 512],
                        start=True, stop=True, perf_mode=DR,
                    )
                if m == 0:
                    # m0 on ACT, m1 on DVE: the first S block needs both, so
                    # they evacuate in parallel
                    nc.scalar.activation(
                        qk_sb[m][:], mm_ps[:], AF.Identity,
                        bias=cst[:, A_QKB + m:A_QKB + m + 1], scale=1.0 / WS,
                    )
                else:
                    nc.vector.tensor_scalar(
                        qk_sb[m][:], mm_ps[:], 1.0 / WS,
                        cst[:, A_QKB + m:A_QKB + m + 1], OP.mult, OP.add,
                    )

            # ---------------- vT (fp8 DoubleRow, ones column per head) -----
            # 272 = 4 heads x 68: the DoubleRow ldweights k-tile stride must be
            # a multiple of 16 elements (s3_lw_dual_fp8_restrictions)
            vt_sb = wpool.tile([128, 8 * 272], F8, name="vt", tag="vt")
            vt4 = vt_sb[:].rearrange("p (e h c) -> p e h c", e=8, h=4, c=68)
            nc.vector.tensor_scalar(
                vt4[:, :, :, 64:65],
                x2[:, 0:32].rearrange("p (a b c) -> p a b c", a=8, b=4),
                0.0, 1.0, OP.mult, OP.add,
            )
            vw = w8[:, 1024:1536].rearrange("p (k c) -> p k c", k=2)
            for et in range(8):
                vt_ps = psU.tile([128, C], F32, name="vtp", tag="u")
                nc.tensor.matmul(
                    vt_ps[:], xn3[:, :, et * 128:(et + 1) * 128], vw[:],
                    start=True, stop=True, perf_mode=DR,
                )
                nc.vector.tensor_scalar(
                    vt4[:, et, :, 0:64],
                    vt_ps[:].rearrange("p (h c) -> p h c", h=4),
                    1.0 / WS, None, OP.mult,
                )

            # residual prep (x + proj_bias), off the critical path
            resid2 = wpool.tile([128, 2 * HW], F32, name="resid", tag="resid")
            for t in range(NT):
                nc.vector.tensor_scalar(
                    resid2[:, t * HW:(t + 1) * HW], x2[:, t * HW:(t + 1) * HW],
                    cst[:, A_PB + t:A_PB + t + 1], None, OP.add,
                )

            # ---------------- attention ----------------
            attn2 = wpool.tile([128, 2 * HW], F8, name="attn2", tag="attn2")
            srows = [spool.tile([33, HW], F32, name=f"srows{p}", tag=f"srows{p}")
                     for p in range(2)]
            lnp = [spool.tile([33, HW], F32, name=f"lnp{p}", tag=f"lnp{p}")
                   for p in range(2)]
            srecr = [spool.tile([33, HW], F32R, name=f"srecr{p}", tag=f"srecr{p}")
                     for p in range(2)]
            rb_sb = [wpool.tile([128, HW], BF16, name=f"rb{p}", tag=f"rb{p}")
                     for p in range(2)]
            u_ps_pair = [{}, {}]

            # rows 1-31 preset to 1.0: the K=33 rb matmul contracts them
            # against zero selector rows, so they must not be inf/nan
            for p in range(2):
                nc.gpsimd.memset(srows[p][:], 1.0)

            def emit_denom_dve(pair, hh=None):
                # denominator -> 1/s = exp(-ln(s)): DVE reciprocal costs
                # 6.5us/call (iterative ucode) and the custom-DVE approx ops
                # don't codegen on this walrus build, so ACT ln/exp it is -
                # it sits at pair boundaries where the exp stream has slack.
                # hh=0/1 processes one head's row (tail chain: h0's starts
                # while h1's last PV runs); hh=None does both in one pass.
                u_ps = u_ps_pair[pair]
                rows = (0, 1) if hh is None else (hh,)
                for r in rows:
                    nc.vector.tensor_copy(
                        srows[pair][32 * r:32 * r + 1, :], u_ps[r][64:65, :]
                    )
                if hh is None:
                    sl = slice(0, 33)
                else:
                    sl = slice(32 * hh, 32 * hh + 1)
                nc.scalar.activation(lnp[pair][sl, :], srows[pair][sl, :], AF.Ln)
                nc.scalar.activation(
                    srecr[pair][sl, :], lnp[pair][sl, :], AF.Exp, scale=-1.0
                )

            def emit_rb_chain(pair):
                # broadcast 16/s across partitions, then normalize straight
                # out of U's PSUM into fp8 (x16 for proj's fp8 range)
                rb_ps = psM.tile([128, HW], F32, name="rb", tag="ps")
                u_ps = u_ps_pair[pair]
                # per-ch pipeline: ch0's cast+normalize runs while ch1's rb
                # matmul is still on the PE
                for ch in range(2):
                    nc.tensor.matmul(
                        rb_ps[:, ch * 512:(ch + 1) * 512],
                        hsel_r[:],
                        srecr[pair][:, ch * 512:(ch + 1) * 512],
                        start=True, stop=True,
                    )
                    nc.vector.tensor_copy(
                        rb_sb[pair][:, ch * 512:(ch + 1) * 512],
                        rb_ps[:, ch * 512:(ch + 1) * 512],
                    )
                    for hh in range(2):
                        nc.vector.tensor_tensor(
                            attn2[hh * 64:(hh + 1) * 64,
                                  pair * HW + ch * 512:pair * HW + (ch + 1) * 512],
                            u_ps[hh][0:64, ch * 512:(ch + 1) * 512],
                            rb_sb[pair][hh * 64:(hh + 1) * 64, ch * 512:(ch + 1) * 512],
                            OP.mult,
                        )

            for pair in range(2):
                qt = qk_sb[2 * pair]
                kt_t = qk_sb[2 * pair + 1]
                u_ps = u_ps_pair[pair]

                def emit_pv(j, ptv, only_hh=None):
                    for hh in range(2) if only_hh is None else (only_hh,):
                        if hh not in u_ps:
                            u_ps[hh] = psU.tile([65, HW], F32, name=f"u{hh}", tag="u")
                        for ch in range(2):
                            nc.tensor.matmul(
                                u_ps[hh][:, ch * 512:(ch + 1) * 512],
                                vt4[:, 2 * j:2 * j + 2, 2 * pair + hh, 0:65],
                                ptv[:, :, hh, ch, :],
                                start=(j == 0), stop=(j == 3),
                                perf_mode=DR, skip_group_check=True,
                            )

                pending = []
                for j in range(4):
                    pt = ppool.tile([128, 4096], F8, name="p", tag="p")
                    ptv = pt[:].rearrange("p (i h c q) -> p i h c q", i=2, h=2, c=2)
                    for i in range(2):
                        et = 2 * j + i
                        for ch in range(2):
                            s_ps = psM.tile([128, HW], F32, name="s", tag="ps")
                            for hh in range(2):
                                prow = slice(hh * 64, hh * 64 + 64)
                                nc.tensor.matmul(
                                    s_ps[:, hh * 512:(hh + 1) * 512],
                                    kt_t[prow, et * 128:(et + 1) * 128],
                                    qt[prow, ch * 512:(ch + 1) * 512],
                                    start=True, stop=True,
                                )
                            # scale = the 1/8 attention scale left out of q/k
                            nc.scalar.activation(
                                ptv[:, i, 0:2, ch, :], s_ps[:], AF.Exp,
                                bias=cst[:, A_EXPB:A_EXPB + 1], scale=ASCALE,
                            )
                    if pair == 1 and j == 1:
                        # pair0's normalize lands here: after two of pair1's S
                        # blocks (PE is in-order; earlier would stall it on
                        # the DVE reciprocal chain) yet before the first PV
                        # needs pair0's U banks
                        emit_rb_chain(0)
                    if pending:
                        emit_pv(*pending.pop(0))
                    pending.append((j, ptv))
                while pending:
                    j, ptv = pending.pop(0)
                    if pending:
                        emit_pv(j, ptv)
                    elif pair == 1:
                        # final j of the last pair: h0's PVs, then its
                        # denominator chain while h1's PVs still run on the
                        # PE (shortens the serial tail)
                        emit_pv(j, ptv, only_hh=0)
                        emit_denom_dve(pair, hh=0)
                        emit_pv(j, ptv, only_hh=1)
                        emit_denom_dve(pair, hh=1)
                    else:
                        # pair0's rb chain waits for pair1's S blocks anyway;
                        # one combined [33, x] ln/exp costs the ACT stream
                        # less than two per-head passes
                        emit_pv(j, ptv)
                        emit_denom_dve(pair)
            emit_rb_chain(1)

            # ---------------- proj (fp8 DoubleRow) + residual ----------------
            pw = w8[:, 1536:2048].rearrange("p (k c) -> p k c", k=2)
            at3 = attn2[:].rearrange("p (i q) -> p i q", i=2)
            out_sb = wpool.tile([128, 2 * HW], F32, name="out", tag="out")
            for t in range(NT):
                pp = psM.tile([128, HW], F32, name=f"pp{t}", tag="ps")
                for ch in range(2):
                    nc.tensor.matmul(
                        pp[:, ch * 512:(ch + 1) * 512],
                        pw[:, :, t * 128:(t + 1) * 128],
                        at3[:, :, ch * 512:(ch + 1) * 512],
                        start=True, stop=True, perf_mode=DR,
                    )
                nc.vector.scalar_tensor_tensor(
                    out_sb[:, t * HW:(t + 1) * HW], pp[:], 1.0 / 256.0,
                    resid2[:, t * HW:(t + 1) * HW], OP.mult, OP.add,
                )
                nc.sync.dma_start(
                    out=out_ext[:, t * HW:(t + 1) * HW],
                    in_=out_sb[:, t * HW:(t + 1) * HW],
                )
    return nc


# ---------------------------------------------------------------------------
# Host entry point
# ---------------------------------------------------------------------------

_CACHED_NC = None


def _host_inputs(x, gn_gamma, gn_beta, qkv_w, qkv_b, proj_w, proj_b):
    f32 = np.float32
    qw = np.asarray(qkv_w[0:C], f32)
    kw = np.asarray(qkv_w[C:2 * C], f32)
    vw = np.asarray(qkv_w[2 * C:3 * C], f32)
    qb = np.asarray(qkv_b[0:C], f32)
    kb = np.asarray(qkv_b[C:2 * C], f32)
    vb = np.asarray(qkv_b[2 * C:3 * C], f32)
    pw = np.asarray(proj_w, f32)
    pb = np.asarray(proj_b, f32)

    def pack_dr(w):  # [rows, C] -> [128, 2*rows] with [p, kt, row] layout
        r = w.shape[0]
        return (w.T * WS).reshape(2, 128, r).transpose(1, 0, 2).reshape(128, 2 * r)

    w_all = np.concatenate([qw[0:128], kw[0:128], qw[128:256], kw[128:256]], axis=0)
    w8 = np.concatenate(
        [pack_dr(w_all), pack_dr(vw), pack_dr(pw)], axis=1
    ).astype(ml_dtypes.float8_e4m3)  # [128, 2048]

    qkb = np.stack([qb[0:128], kb[0:128], qb[128:256], kb[128:256]])
    pb_eff = (pb + pw @ vb).reshape(2, 128)
    gam = np.asarray(gn_gamma, f32).reshape(2, 128)
    bet = np.asarray(gn_beta, f32).reshape(2, 128)

    cst = np.zeros((128, CSTW), f32)
    cst[:, A_QKB:A_QKB + 4] = qkb.T
    cst[:, A_PB:A_PB + 2] = pb_eff.T
    cst[:, A_GAM:A_GAM + 2] = gam.T
    cst[:, A_BET:A_BET + 2] = bet.T
    cidx = np.arange(C)
    gsel = (cidx[:, None] // CPG == np.arange(GROUPS)[None, :]).astype(f32) / (CPG * HW)
    cst[:, A_GSEL:A_GSEL + 8] = gsel[0:128]
    cst[:, A_GSEL + 8:A_GSEL + 16] = gsel[128:256]
    cst[:, A_EXPB] = -SHIFT
    cst[0, A_HSEL:A_HSEL + 64] = WS
    cst[32, A_HSEL + 64:A_HSEL + 128] = WS
    mask = (cidx[:, None] // CPG == np.arange(GROUPS)[None, :]).astype(f32)
    gamv = np.asarray(gn_gamma, f32)
    cst[0:8, A_GSC:A_GSC + 128] = (mask[0:128] * gamv[0:128, None]).T
    cst[32:40, A_GSC:A_GSC + 128] = (mask[128:256] * gamv[128:256, None]).T

    shared = {"w8": w8, "cst": np.ascontiguousarray(cst)}
    x = np.asarray(x, f32)
    in_maps = []
    for i in range(N_CORES):
        m = dict(shared)
        m["x"] = np.ascontiguousarray(
            x[i].reshape(2, 128, HW).transpose(1, 0, 2).reshape(128, 2 * HW)
        )
        in_maps.append(m)
    return in_maps


def kernel(x, gn_gamma, gn_beta, qkv_w, qkv_b, proj_w, proj_b):
    global _CACHED_NC
    _install_patches()
    in_maps = _host_inputs(x, gn_gamma, gn_beta, qkv_w, qkv_b, proj_w, proj_b)
    if _CACHED_NC is None:
        _CACHED_NC = build_nc()
    res = run_bass_kernel_spmd(_CACHED_NC, in_maps, core_ids=list(range(N_CORES)))
    out = np.stack([
        res.results[i]["out"].reshape(128, 2, HW).transpose(1, 0, 2).reshape(C, 32, 32)
        for i in range(N_CORES)
    ])
    return out.astype(np.float32)
